# revision 15
# baseline (speedup 1.0000x reference)
"""Multi-head attention (b=2, s=2048, d=1024, 16 heads) on 8 trn2 cores.

Sharding: core c -> batch c//4, head-group c%4 (4 heads each).
Data-parallel over batch, tensor-parallel over heads; the 4 partial
output projections per batch are summed on the host (the TP all-reduce).

Per-core program (matmuls in bf16, fp32 PSUM accumulation):
  qkT [512,2048]  = wqkT.T @ xT          (+ bias, per-partition)
  V   [2048,4,65] = x @ wv (+ bias), augmented with a ones column
  heads processed in pairs (row-group-concurrent scores matmuls);
  per pair, query-chunk qc (512 wide), key-chunk pair kc2:
    sT(kc)   = kT(kc-chunk).T @ qT       -> PSUM [128,1024] per head
    E        = exp(0.125 * sT)           -> ACT, PSUM->SBUF bf16
    out_aug += V_aug(kc).T @ E           -> PSUM [65,512]; row 64 = denom
    OT       = out_aug[0:64] / denom     (DVE divide eviction)
  yT [1024,2048] = projT.T @ OT          (partial; host sums over 4 cores)
"""

import numpy as np

N_CORES = 8
P = 128
S = 2048
D = 1024
HD = 64
NH = 4        # heads per core
SCALE = HD ** -0.5
KC = S // P   # 16 key chunks
QC = 4        # query chunks
NQ = S // QC  # 512
KD = D // P   # 8 contraction chunks for d=1024

_CACHE = {}


def build_program():
    import contextlib

    import concourse.mybir as mybir
    import concourse.tile as tile
    from concourse import bacc

    F32 = mybir.dt.float32
    BF16 = mybir.dt.bfloat16
    Exp = mybir.ActivationFunctionType.Exp

    nc = bacc.Bacc("TRN2", target_bir_lowering=False, debug=False,
                   num_devices=N_CORES)

    xT = nc.dram_tensor("xT", [D, S], BF16, kind="ExternalInput").ap()
    wqkT = nc.dram_tensor("wqkT", [D, 512], BF16, kind="ExternalInput").ap()
    bqk = nc.dram_tensor("bqk", [P, 4], F32, kind="ExternalInput").ap()
    wv = nc.dram_tensor("wv", [D, 256], BF16, kind="ExternalInput").ap()
    bvb = nc.dram_tensor("bvb", [P, 256], F32, kind="ExternalInput").ap()
    projT = nc.dram_tensor("projT", [256, D], BF16, kind="ExternalInput").ap()
    yT = nc.dram_tensor("yT", [D, S], F32, kind="ExternalOutput").ap()

    xT_r = xT.rearrange("(ko p) s -> p ko s", p=P)       # [128, 8, 2048]
    wqkT_r = wqkT.rearrange("(ko p) m -> p ko m", p=P)   # [128, 8, 512]
    wv_r = wv.rearrange("(ko p) m -> p ko m", p=P)       # [128, 8, 256]
    projT_r = projT.rearrange("(ko p) m -> p ko m", p=P)  # [128, 2, 1024]
    yT_r = yT.rearrange("(mo p) s -> p mo s", p=P)       # [128, 8, 2048]

    with tile.TileContext(nc) as tc:
        ctx = contextlib.ExitStack()
        with ctx:
            const = ctx.enter_context(tc.tile_pool(name="const", bufs=1))
            xt_pool = ctx.enter_context(tc.tile_pool(name="xt", bufs=1))
            qk_pool = ctx.enter_context(tc.tile_pool(name="qk", bufs=1))
            v_pool = ctx.enter_context(tc.tile_pool(name="v", bufs=1))
            ot_pool = ctx.enter_context(tc.tile_pool(name="ot", bufs=1))
            e_pool = ctx.enter_context(tc.tile_pool(name="e", bufs=4))
            rb_pool = ctx.enter_context(tc.tile_pool(name="rb", bufs=2))
            # PSUM: [128,1024] slots (2 banks each) shared by scores / qk /
            # V / proj, double-buffered; plus double-buffered accumulators
            # per head of the active pair.  2*2 + 2*1 + 2*1 = 8 banks.
            ps_pool = ctx.enter_context(
                tc.tile_pool(name="ps", bufs=2, space="PSUM"))
            ps_oa = ctx.enter_context(
                tc.tile_pool(name="ps_oa", bufs=2, space="PSUM"))

            # ---- constant loads ----
            wqk_sb = const.tile([P, KD, 512], BF16)
            nc.sync.dma_start(out=wqk_sb[:], in_=wqkT_r)
            wv_sb = const.tile([P, KD, 256], BF16)
            nc.sync.dma_start(out=wv_sb[:], in_=wv_r)
            bqk_sb = const.tile([P, 4], F32)
            nc.sync.dma_start(out=bqk_sb[:], in_=bqk)
            bvb_sb = const.tile([P, 4, HD], F32)
            nc.sync.dma_start(out=bvb_sb[:], in_=bvb.rearrange(
                "p (h d) -> p h d", d=HD))
            projT_sb = const.tile([P, 2, D], BF16)
            nc.sync.dma_start(out=projT_sb[:], in_=projT_r)

            xt_sb = []
            for k in range(KD):
                t = xt_pool.tile([P, S], BF16, name=f"xt{k}")
                nc.sync.dma_start(out=t[:], in_=xT_r[:, k, :])
                xt_sb.append(t)

            # persistent result tiles
            qk_sb = [qk_pool.tile([P, S], BF16, name=f"qk{m}")
                     for m in range(4)]
            V_sb = v_pool.tile([P, KC, NH, HD + 1], BF16)
            ot_sb = [ot_pool.tile([P, S], BF16, name=f"ot{k}")
                     for k in range(2)]

            ones_sb = const.tile([P, 1], F32)
            nc.vector.memset(ones_sb[:], 1.0)
            nc.vector.tensor_copy(
                V_sb[:, :, :, HD:HD + 1],
                ones_sb[:, None, None, :].broadcast_to([P, KC, NH, 1]))

            # ---- qkT projection chunks ----
            def qk_chunk(m):
                for n in range(QC):
                    ps = ps_pool.tile([P, 2 * NQ], F32, name="ps")
                    for k in range(KD):
                        nc.tensor.matmul(
                            ps[:, 0:NQ],
                            lhsT=wqk_sb[:, k, m * P:(m + 1) * P],
                            rhs=xt_sb[k][:, n * NQ:(n + 1) * NQ],
                            start=(k == 0), stop=(k == KD - 1))
                    nc.vector.tensor_scalar_add(
                        qk_sb[m][:, n * NQ:(n + 1) * NQ], ps[:, 0:NQ],
                        bqk_sb[:, m:m + 1])

            qk_chunk(0)
            qk_chunk(2)

            # ---- V (natural layout) + bias, into augmented tile ----
            for mk in range(KC):
                ps = ps_pool.tile([P, 2 * NQ], F32, name="ps")
                for k in range(KD):
                    nc.tensor.matmul(
                        ps[:, 0:256],
                        lhsT=xt_sb[k][:, mk * P:(mk + 1) * P],
                        rhs=wv_sb[:, k, :],
                        start=(k == 0), stop=(k == KD - 1))
                nc.vector.tensor_add(
                    V_sb[:, mk, :, 0:HD],
                    ps[:, 0:256].rearrange("p (h d) -> p h d", d=HD),
                    bvb_sb[:])

            # ---- output projection for one query chunk (partial) ----
            y_pool = ctx.enter_context(tc.tile_pool(name="y", bufs=3))

            def proj_chunk(n):
                for m in range(KD):
                    ps = ps_pool.tile([P, 2 * NQ], F32, name="ps")
                    for k in range(2):
                        nc.tensor.matmul(
                            ps[:, 0:NQ],
                            lhsT=projT_sb[:, k, m * P:(m + 1) * P],
                            rhs=ot_sb[k][:, n * NQ:(n + 1) * NQ],
                            start=(k == 0), stop=(k == 1))
                    yt = y_pool.tile([P, NQ], F32, name="yt")
                    if m % 2 == 0:
                        nc.vector.tensor_copy(yt[:], ps[:, 0:NQ])
                    else:
                        nc.scalar.copy(yt[:], ps[:, 0:NQ])
                    nc.sync.dma_start(
                        out=yT_r[:, m, n * NQ:(n + 1) * NQ], in_=yt[:])

            # ---- attention, head pair (h0, h0+1), query chunks in blocks
            # of two so the ACT queue always holds several independent exps
            def attention_pair(h0, with_proj=False):
                qt = qk_sb[h0 // 2]
                kt = qk_sb[2 + h0 // 2]
                for qc0 in range(0, QC, 2):
                    oa = [[ps_oa.tile([P, NQ], F32, name=f"oa{i}{qq}",
                                      bufs=1)
                           for i in range(2)] for qq in range(2)]
                    for kc2 in range(KC // 2):
                        for qq in range(2):
                            qc = qc0 + qq
                            sc = [ps_pool.tile([P, 2 * NQ], F32, name="ps")
                                  for _ in range(2)]
                            # scores: two heads on disjoint PE row groups
                            for j in range(2):
                                kc = kc2 * 2 + j
                                for i in range(2):
                                    qb = HD * i
                                    nc.tensor.matmul(
                                        sc[i][:, j * NQ:(j + 1) * NQ],
                                        lhsT=kt[qb:qb + HD,
                                                kc * P:(kc + 1) * P],
                                        rhs=qt[qb:qb + HD,
                                               qc * NQ:(qc + 1) * NQ],
                                        start=True, stop=True)
                            es = []
                            for i in range(2):
                                e = e_pool.tile([P, 2 * NQ], BF16, name="e")
                                nc.scalar.activation(e[:], sc[i][:], Exp,
                                                     scale=SCALE)
                                es.append(e)
                            for j in range(2):
                                kc = kc2 * 2 + j
                                for i in range(2):
                                    nc.tensor.matmul(
                                        oa[qq][i][0:HD + 1, :],
                                        lhsT=V_sb[:, kc, h0 + i, :],
                                        rhs=es[i][:, j * NQ:(j + 1) * NQ],
                                        start=(kc == 0), stop=(kc == KC - 1))
                    for qq in range(2):
                        qc = qc0 + qq
                        for i in range(2):
                            h = h0 + i
                            dens = rb_pool.tile([1, NQ], F32, name="dens")
                            nc.vector.tensor_copy(dens[:],
                                                  oa[qq][i][HD:HD + 1, :])
                            recs = rb_pool.tile([1, NQ], F32, name="recs")
                            nc.vector.reciprocal_approx_fast(recs[:],
                                                             dens[:])
                            denb = rb_pool.tile([HD, NQ], F32, name="denb")
                            nc.gpsimd.partition_broadcast(denb[:], recs[:])
                            nc.vector.tensor_mul(
                                ot_sb[h // 2][HD * (h % 2):HD * (h % 2) + HD,
                                              qc * NQ:(qc + 1) * NQ],
                                oa[qq][i][0:HD, :], denb[:])
                    if with_proj:
                        for qq in range(2):
                            proj_chunk(qc0 + qq)

            attention_pair(0)
            # remaining qkT chunks fill PE slack under ACT-bound attention
            qk_chunk(1)
            qk_chunk(3)
            attention_pair(2, with_proj=True)

    nc.compile()
    return nc


def get_program():
    if "nc" not in _CACHE:
        _CACHE["nc"] = build_program()
    return _CACHE["nc"]


def _bf16(a):
    import ml_dtypes

    return np.ascontiguousarray(a, np.float32).astype(ml_dtypes.bfloat16)


def shard_inputs(x, qkv_w, qkv_b, proj_w):
    """Per-core input maps. Core c: batch c//4, head group g=c%4."""
    x = np.asarray(x, np.float32)
    qkv_w = np.asarray(qkv_w, np.float32)
    qkv_b = np.asarray(qkv_b, np.float32)
    proj_w = np.asarray(proj_w, np.float32)
    in_maps = []
    for c in range(N_CORES):
        b, g = divmod(c, 4)
        r0 = g * 256
        q_w = qkv_w[r0:r0 + 256]               # [256, 1024]
        k_w = qkv_w[D + r0:D + r0 + 256]
        v_w = qkv_w[2 * D + r0:2 * D + r0 + 256]
        wqkT = _bf16(np.concatenate([q_w, k_w], 0).T)   # [1024, 512]
        bqk_c = np.concatenate([qkv_b[r0:r0 + 256],
                                qkv_b[D + r0:D + r0 + 256]])
        bqk = np.ascontiguousarray(bqk_c.reshape(4, P).T)   # [128, 4]
        wv = _bf16(v_w.T)                      # [1024, 256]
        bv = qkv_b[2 * D + r0:2 * D + r0 + 256]
        bvb = np.ascontiguousarray(
            np.broadcast_to(bv, (P, 256)))     # [128, 256]
        projT = _bf16(proj_w[:, r0:r0 + 256].T)  # [256, 1024]
        in_maps.append({
            "xT": _bf16(x[b].T),
            "wqkT": wqkT,
            "bqk": bqk,
            "wv": wv,
            "bvb": bvb,
            "projT": projT,
        })
    return in_maps


def unshard_output(results, proj_b):
    out = np.empty((2, S, D), np.float32)
    for b in range(2):
        acc = results[4 * b]["yT"].copy()
        for g in range(1, 4):
            acc += results[4 * b + g]["yT"]
        out[b] = acc.T + np.asarray(proj_b, np.float32)[None, :]
    return out


def kernel(x, qkv_w, qkv_b, proj_w, proj_b):
    from concourse.bass_utils import run_bass_kernel_spmd

    nc = get_program()
    in_maps = shard_inputs(x, qkv_w, qkv_b, proj_w)
    res = run_bass_kernel_spmd(nc, in_maps, core_ids=list(range(N_CORES)))
    return unshard_output(res.results, proj_b)


# revision 20
# speedup vs baseline: 1.0424x; 1.0424x over previous
"""Multi-head attention (b=2, s=2048, d=1024, 16 heads) on 8 trn2 cores.

Sharding: core c -> batch c//4, head-group c%4 (4 heads each).
Data-parallel over batch, tensor-parallel over heads; the 4 partial
output projections per batch are summed on the host (the TP all-reduce).

Per-core program (matmuls in bf16, fp32 PSUM accumulation):
  qkT [512,2048]  = wqkT.T @ xT          (+ bias, per-partition)
  V   [2048,4,65] = x @ wv (+ bias), augmented with a ones column
  heads processed in pairs (row-group-concurrent scores matmuls);
  per pair, query-chunk qc (512 wide), key-chunk pair kc2:
    sT(kc)   = kT(kc-chunk).T @ qT       -> PSUM [128,1024] per head
    E        = exp(0.125 * sT)           -> ACT, PSUM->SBUF bf16
    out_aug += V_aug(kc).T @ E           -> PSUM [65,512]; row 64 = denom
    OT       = out_aug[0:64] / denom     (DVE divide eviction)
  yT [1024,2048] = projT.T @ OT          (partial; host sums over 4 cores)
"""

import numpy as np

N_CORES = 8
P = 128
S = 2048
D = 1024
HD = 64
NH = 4        # heads per core
SCALE = HD ** -0.5
KC = S // P   # 16 key chunks
QC = 4        # query chunks
NQ = S // QC  # 512
KD = D // P   # 8 contraction chunks for d=1024

_CACHE = {}


def build_program():
    import contextlib

    import concourse.mybir as mybir
    import concourse.tile as tile
    from concourse import bacc

    F32 = mybir.dt.float32
    BF16 = mybir.dt.bfloat16
    Exp = mybir.ActivationFunctionType.Exp

    nc = bacc.Bacc("TRN2", target_bir_lowering=False, debug=False,
                   num_devices=N_CORES)

    xT = nc.dram_tensor("xT", [D, S], BF16, kind="ExternalInput").ap()
    wqkT = nc.dram_tensor("wqkT", [D, 512], BF16, kind="ExternalInput").ap()
    bqk = nc.dram_tensor("bqk", [P, 4], F32, kind="ExternalInput").ap()
    wv = nc.dram_tensor("wv", [D, 256], BF16, kind="ExternalInput").ap()
    bvb = nc.dram_tensor("bvb", [P, 256], F32, kind="ExternalInput").ap()
    projT = nc.dram_tensor("projT", [256, D], BF16, kind="ExternalInput").ap()
    yT = nc.dram_tensor("yT", [D, S], F32, kind="ExternalOutput").ap()

    xT_r = xT.rearrange("(ko p) s -> p ko s", p=P)       # [128, 8, 2048]
    wqkT_r = wqkT.rearrange("(ko p) m -> p ko m", p=P)   # [128, 8, 512]
    wv_r = wv.rearrange("(ko p) m -> p ko m", p=P)       # [128, 8, 256]
    projT_r = projT.rearrange("(ko p) m -> p ko m", p=P)  # [128, 2, 1024]
    yT_r = yT.rearrange("(mo p) s -> p mo s", p=P)       # [128, 8, 2048]

    with tile.TileContext(nc) as tc:
        ctx = contextlib.ExitStack()
        with ctx:
            const = ctx.enter_context(tc.tile_pool(name="const", bufs=1))
            xt_pool = ctx.enter_context(tc.tile_pool(name="xt", bufs=1))
            qk_pool = ctx.enter_context(tc.tile_pool(name="qk", bufs=1))
            v_pool = ctx.enter_context(tc.tile_pool(name="v", bufs=1))
            ot_pool = ctx.enter_context(tc.tile_pool(name="ot", bufs=1))
            e_pool = ctx.enter_context(tc.tile_pool(name="e", bufs=4))
            rb_pool = ctx.enter_context(tc.tile_pool(name="rb", bufs=2))
            # PSUM: [128,1024] slots (2 banks each) shared by scores / qk /
            # V / proj, double-buffered; plus double-buffered accumulators
            # per head of the active pair.  2*2 + 2*1 + 2*1 = 8 banks.
            ps_pool = ctx.enter_context(
                tc.tile_pool(name="ps", bufs=2, space="PSUM"))
            ps_oa = ctx.enter_context(
                tc.tile_pool(name="ps_oa", bufs=2, space="PSUM"))

            # ---- constant loads ----
            wqk_sb = const.tile([P, KD, 512], BF16)
            nc.sync.dma_start(out=wqk_sb[:], in_=wqkT_r)
            wv_sb = const.tile([P, KD, 256], BF16)
            nc.sync.dma_start(out=wv_sb[:], in_=wv_r)
            bqk_sb = const.tile([P, 4], F32)
            nc.sync.dma_start(out=bqk_sb[:], in_=bqk)
            bvb_sb = const.tile([P, 4, HD], F32)
            nc.sync.dma_start(out=bvb_sb[:], in_=bvb.rearrange(
                "p (h d) -> p h d", d=HD))
            projT_sb = const.tile([P, 2, D], BF16)
            nc.sync.dma_start(out=projT_sb[:], in_=projT_r)

            xt_sb = []
            for k in range(KD):
                t = xt_pool.tile([P, S], BF16, name=f"xt{k}")
                nc.sync.dma_start(out=t[:], in_=xT_r[:, k, :])
                xt_sb.append(t)

            # persistent result tiles
            qk_sb = [qk_pool.tile([P, S], BF16, name=f"qk{m}")
                     for m in range(4)]
            V_sb = v_pool.tile([P, KC, NH, HD + 1], BF16)
            ot_sb = [ot_pool.tile([P, S], BF16, name=f"ot{k}")
                     for k in range(2)]

            ones_sb = const.tile([P, 1], F32)
            nc.vector.memset(ones_sb[:], 1.0)
            nc.vector.tensor_copy(
                V_sb[:, :, :, HD:HD + 1],
                ones_sb[:, None, None, :].broadcast_to([P, KC, NH, 1]))

            # ---- qkT projection, one query/key chunk ----
            def qk_chunk(m, n):
                ps = ps_pool.tile([P, 2 * NQ], F32, name="ps")
                for k in range(KD):
                    nc.tensor.matmul(
                        ps[:, 0:NQ],
                        lhsT=wqk_sb[:, k, m * P:(m + 1) * P],
                        rhs=xt_sb[k][:, n * NQ:(n + 1) * NQ],
                        start=(k == 0), stop=(k == KD - 1))
                nc.vector.tensor_scalar_add(
                    qk_sb[m][:, n * NQ:(n + 1) * NQ], ps[:, 0:NQ],
                    bqk_sb[:, m:m + 1])

            # ---- V (natural layout) + bias, one key chunk ----
            def v_chunk(mk):
                ps = ps_pool.tile([P, 2 * NQ], F32, name="ps")
                for k in range(KD):
                    nc.tensor.matmul(
                        ps[:, 0:256],
                        lhsT=xt_sb[k][:, mk * P:(mk + 1) * P],
                        rhs=wv_sb[:, k, :],
                        start=(k == 0), stop=(k == KD - 1))
                nc.vector.tensor_add(
                    V_sb[:, mk, :, 0:HD],
                    ps[:, 0:256].rearrange("p (h d) -> p h d", d=HD),
                    bvb_sb[:])

            # ---- output projection for one query chunk (partial) ----
            y_pool = ctx.enter_context(tc.tile_pool(name="y", bufs=3))

            def proj_chunk(n):
                for m in range(KD):
                    ps = ps_pool.tile([P, 2 * NQ], F32, name="ps")
                    for k in range(2):
                        nc.tensor.matmul(
                            ps[:, 0:NQ],
                            lhsT=projT_sb[:, k, m * P:(m + 1) * P],
                            rhs=ot_sb[k][:, n * NQ:(n + 1) * NQ],
                            start=(k == 0), stop=(k == 1))
                    yt = y_pool.tile([P, NQ], F32, name="yt")
                    if m % 2 == 0:
                        nc.vector.tensor_copy(yt[:], ps[:, 0:NQ])
                    else:
                        nc.scalar.copy(yt[:], ps[:, 0:NQ])
                    nc.sync.dma_start(
                        out=yT_r[:, m, n * NQ:(n + 1) * NQ], in_=yt[:])

            # ---- attention, head pair (h0, h0+1) ----
            # feed_kc2/feed_qc: extra work emitted at the bottom of the
            # given (qc==0) kc2 iteration / qc iteration, one step before
            # the attention instructions that consume its results, so the
            # scheduler can slot it into PE gaps of the ACT-bound pipeline.
            def attention_pair(h0, with_proj=False, feed_kc2=None,
                               feed_qc=None):
                qt = qk_sb[h0 // 2]
                kt = qk_sb[2 + h0 // 2]
                for qc in range(QC):
                    oa = [ps_oa.tile([P, NQ], F32, name=f"oa{i}")
                          for i in range(2)]
                    for kc2 in range(KC // 2):
                        sc = [ps_pool.tile([P, 2 * NQ], F32, name="ps")
                              for _ in range(2)]
                        # scores: two heads on disjoint PE row groups
                        for j in range(2):
                            kc = kc2 * 2 + j
                            for i in range(2):
                                qb = HD * i
                                nc.tensor.matmul(
                                    sc[i][:, j * NQ:(j + 1) * NQ],
                                    lhsT=kt[qb:qb + HD, kc * P:(kc + 1) * P],
                                    rhs=qt[qb:qb + HD, qc * NQ:(qc + 1) * NQ],
                                    start=True, stop=True)
                        es = []
                        for i in range(2):
                            e = e_pool.tile([P, 2 * NQ], BF16, name="e")
                            nc.scalar.activation(e[:], sc[i][:], Exp,
                                                 scale=SCALE)
                            es.append(e)
                        for j in range(2):
                            kc = kc2 * 2 + j
                            for i in range(2):
                                nc.tensor.matmul(
                                    oa[i][0:HD + 1, :],
                                    lhsT=V_sb[:, kc, h0 + i, :],
                                    rhs=es[i][:, j * NQ:(j + 1) * NQ],
                                    start=(kc == 0), stop=(kc == KC - 1))
                        if qc == 0 and feed_kc2 and kc2 in feed_kc2:
                            for thunk in feed_kc2[kc2]:
                                thunk()
                    for i in range(2):
                        h = h0 + i
                        dens = rb_pool.tile([1, NQ], F32, name="dens")
                        nc.vector.tensor_copy(dens[:], oa[i][HD:HD + 1, :])
                        recs = rb_pool.tile([1, NQ], F32, name="recs")
                        nc.vector.reciprocal_approx_fast(recs[:], dens[:])
                        denb = rb_pool.tile([HD, NQ], F32, name="denb")
                        nc.gpsimd.partition_broadcast(denb[:], recs[:])
                        nc.vector.tensor_mul(
                            ot_sb[h // 2][HD * (h % 2):HD * (h % 2) + HD,
                                          qc * NQ:(qc + 1) * NQ],
                            oa[i][0:HD, :], denb[:])
                    if with_proj:
                        proj_chunk(qc)
                    if feed_qc and qc in feed_qc:
                        for thunk in feed_qc[qc]:
                            thunk()

            # lead-in: just enough projection work to light up attention;
            # the rest is fed in one step ahead of its consumers so the
            # scheduler slots it into PE gaps of the ACT-bound pipeline.
            qk_chunk(2, 0)
            qk_chunk(0, 0)
            v_chunk(0)
            v_chunk(1)
            feed_kc2 = {
                0: [lambda: v_chunk(2), lambda: v_chunk(3)],
                1: [lambda: v_chunk(4), lambda: v_chunk(5),
                    lambda: qk_chunk(2, 1)],
                2: [lambda: v_chunk(6), lambda: v_chunk(7)],
                3: [lambda: v_chunk(8), lambda: v_chunk(9),
                    lambda: qk_chunk(2, 2)],
                4: [lambda: v_chunk(10), lambda: v_chunk(11)],
                5: [lambda: v_chunk(12), lambda: v_chunk(13),
                    lambda: qk_chunk(2, 3)],
                6: [lambda: v_chunk(14), lambda: v_chunk(15)],
            }
            feed_qc = {
                0: [lambda: qk_chunk(0, 1)],
                1: [lambda: qk_chunk(0, 2)],
                2: [lambda: qk_chunk(0, 3)],
            }
            attention_pair(0, feed_kc2=feed_kc2, feed_qc=feed_qc)
            for n in range(QC):
                qk_chunk(1, n)
                qk_chunk(3, n)
            attention_pair(2, with_proj=True)

    nc.compile()
    return nc


def get_program():
    if "nc" not in _CACHE:
        _CACHE["nc"] = build_program()
    return _CACHE["nc"]


def _bf16(a):
    import ml_dtypes

    return np.ascontiguousarray(a, np.float32).astype(ml_dtypes.bfloat16)


def shard_inputs(x, qkv_w, qkv_b, proj_w):
    """Per-core input maps. Core c: batch c//4, head group g=c%4."""
    x = np.asarray(x, np.float32)
    qkv_w = np.asarray(qkv_w, np.float32)
    qkv_b = np.asarray(qkv_b, np.float32)
    proj_w = np.asarray(proj_w, np.float32)
    in_maps = []
    for c in range(N_CORES):
        b, g = divmod(c, 4)
        r0 = g * 256
        q_w = qkv_w[r0:r0 + 256]               # [256, 1024]
        k_w = qkv_w[D + r0:D + r0 + 256]
        v_w = qkv_w[2 * D + r0:2 * D + r0 + 256]
        wqkT = _bf16(np.concatenate([q_w, k_w], 0).T)   # [1024, 512]
        bqk_c = np.concatenate([qkv_b[r0:r0 + 256],
                                qkv_b[D + r0:D + r0 + 256]])
        bqk = np.ascontiguousarray(bqk_c.reshape(4, P).T)   # [128, 4]
        wv = _bf16(v_w.T)                      # [1024, 256]
        bv = qkv_b[2 * D + r0:2 * D + r0 + 256]
        bvb = np.ascontiguousarray(
            np.broadcast_to(bv, (P, 256)))     # [128, 256]
        projT = _bf16(proj_w[:, r0:r0 + 256].T)  # [256, 1024]
        in_maps.append({
            "xT": _bf16(x[b].T),
            "wqkT": wqkT,
            "bqk": bqk,
            "wv": wv,
            "bvb": bvb,
            "projT": projT,
        })
    return in_maps


def unshard_output(results, proj_b):
    out = np.empty((2, S, D), np.float32)
    for b in range(2):
        acc = results[4 * b]["yT"].copy()
        for g in range(1, 4):
            acc += results[4 * b + g]["yT"]
        out[b] = acc.T + np.asarray(proj_b, np.float32)[None, :]
    return out


def kernel(x, qkv_w, qkv_b, proj_w, proj_b):
    from concourse.bass_utils import run_bass_kernel_spmd

    nc = get_program()
    in_maps = shard_inputs(x, qkv_w, qkv_b, proj_w)
    res = run_bass_kernel_spmd(nc, in_maps, core_ids=list(range(N_CORES)))
    return unshard_output(res.results, proj_b)
